# revision 3
# baseline (speedup 1.0000x reference)
"""Trainium2 Bass kernel for nn_Attention_67156108640667 (pooling attention).

reference:
    energies = einsum('btd,d->bt', x, v)         # GEMV per batch
    weights  = softmax(energies, axis=1)          # [B, T]
    context  = einsum('bt,btd->bd', weights, x)   # weighted-sum pool

Shapes: B=64, T=4096, D=512, f32.

Strategy (data-parallel over B across 8 NeuronCores, no collectives):
  Per core: 8 batches, each x[b] = [4096, 512] f32 (8 MiB).  Single HBM pass:
  - Stream x in 1 MiB chunks; for each [128t, 512d] tile one fused DVE
    tensor_tensor_reduce computes prod = x*v_bcast (written bf16, kept for
    the context matmul) and accum_out = energies column [128, 1].
  - Softmax on the [128, 32] energy matrix per batch (PE transposes/broadcasts
    for the cross-partition max/sum, ACT exp with accumulate).
  - Context = sum_t w[t] * prod[t, :] via PE bf16 matmuls accumulated in PSUM,
    then divided by v at the end (prod/v == x with only bf16 relative error).
"""

import os
from contextlib import ExitStack

import numpy as np

import concourse.bass as bass
import concourse.bacc as bacc
import concourse.tile as tile
from concourse import mybir
from concourse import bass_utils
from concourse.masks import make_identity

B, T, D = 64, 4096, 512
NCORES = 8
BPC = B // NCORES            # 8 batches per core
P = 128                      # SBUF partitions
NT = T // P                  # 32 t-tiles per batch
CHUNK_TILES = 4              # t-tiles per DMA chunk (4*128*512*4B = 1 MiB)
NCHUNK = NT // CHUNK_TILES   # 8 chunks per batch

F32 = mybir.dt.float32
BF16 = mybir.dt.bfloat16
ALU = mybir.AluOpType
ACTF = mybir.ActivationFunctionType
AX = mybir.AxisListType


def build_tile_kernel(tc, x_ap, v_ap, ctx_ap, w_ap):
    """Emit the per-core program.

    x_ap:   [BPC, T, D] f32 DRAM in
    v_ap:   [D, 1]      f32 DRAM in
    ctx_ap: [BPC, D]    f32 DRAM out
    w_ap:   [BPC, T]    f32 DRAM out
    """
    nc = tc.nc
    with ExitStack() as ctx:
        consts = ctx.enter_context(tc.tile_pool(name="consts", bufs=1))
        xpool = ctx.enter_context(tc.tile_pool(name="xpool", bufs=8))
        prodp = ctx.enter_context(tc.tile_pool(name="prodp", bufs=2))
        statp = ctx.enter_context(tc.tile_pool(name="statp", bufs=3))
        outp = ctx.enter_context(tc.tile_pool(name="outp", bufs=3))
        ps_small = ctx.enter_context(tc.tile_pool(name="ps_small", bufs=3, space="PSUM"))
        ps_w = ctx.enter_context(tc.tile_pool(name="ps_w", bufs=2, space="PSUM"))
        ps_ctx = ctx.enter_context(tc.tile_pool(name="ps_ctx", bufs=2, space="PSUM"))

        # ---- one-time constants ----
        ident = consts.tile([P, P], F32)
        make_identity(nc, ident)
        ones_row = consts.tile([1, P], F32)
        nc.vector.memset(ones_row, 1.0)
        neg_ones_row = consts.tile([1, P], F32)
        nc.vector.memset(neg_ones_row, -1.0)
        ones_col = consts.tile([P, 1], F32)
        nc.vector.memset(ones_col, 1.0)

        v_row_src = v_ap.rearrange("d one -> one d")  # [1, D] in DRAM
        v_row = consts.tile([1, D], F32)
        nc.sync.dma_start(out=v_row, in_=v_row_src)
        v_bcast = consts.tile([P, D], F32)
        nc.gpsimd.dma_start(out=v_bcast, in_=v_row_src.to_broadcast([P, D]))
        v_inv = consts.tile([1, D], F32)
        nc.vector.reciprocal(v_inv, v_row)

        for b in range(BPC):
            # ---- stage 1: stream x, fused multiply + row-reduce (energies) --
            E_cols = statp.tile([P, NT], F32, tag="E_cols")
            prod_b = prodp.tile([P, NT, D], BF16, tag="prod")
            for c in range(NCHUNK):
                xc = xpool.tile([P, CHUNK_TILES, D], F32, tag="xc")
                rows = x_ap[b, c * CHUNK_TILES * P : (c + 1) * CHUNK_TILES * P, :]
                nc.sync.dma_start(out=xc, in_=rows.rearrange("(j p) d -> p j d", p=P))
                for jj in range(CHUNK_TILES):
                    j = c * CHUNK_TILES + jj
                    nc.vector.scalar_tensor_tensor(
                        out=prod_b[:, j, :],
                        in0=xc[:, jj, :],
                        scalar=1.0,
                        in1=v_bcast,
                        op0=ALU.mult,
                        op1=ALU.mult,
                        accum_out=E_cols[:, j : j + 1],
                    )

            # ---- stage 2: softmax over all T=4096 energies of batch b ------
            m1 = statp.tile([P, 1], F32, tag="m1")
            nc.vector.tensor_reduce(m1, E_cols, axis=AX.X, op=ALU.max)
            mT_ps = ps_small.tile([1, P], F32, tag="ps_small")
            nc.tensor.transpose(mT_ps, m1, ident)
            mT = statp.tile([1, P], F32, tag="mT")
            nc.scalar.copy(mT, mT_ps)
            # global max m, broadcast of -m to all 128 partitions via PE
            m = statp.tile([1, 1], F32, tag="m")
            nc.vector.tensor_reduce(m, mT, axis=AX.X, op=ALU.max)
            negm_ps = ps_small.tile([P, 1], F32, tag="ps_small")
            nc.tensor.matmul(negm_ps, lhsT=neg_ones_row, rhs=m, start=True, stop=True)
            negm = statp.tile([P, 1], F32, tag="negm")
            nc.scalar.copy(negm, negm_ps)
            # P_cols = exp(E - m), s1 = per-partition sums
            P_cols = statp.tile([P, NT], F32, tag="P_cols")
            s1 = statp.tile([P, 1], F32, tag="s1")
            nc.scalar.activation(
                P_cols, E_cols, ACTF.Exp, bias=negm, scale=1.0, accum_out=s1
            )
            # total sum across partitions, reciprocal, broadcast back
            s_ps = ps_small.tile([1, 1], F32, tag="ps_small")
            nc.tensor.matmul(s_ps, lhsT=s1, rhs=ones_col, start=True, stop=True)
            s_sb = statp.tile([1, 1], F32, tag="s_sb")
            nc.scalar.copy(s_sb, s_ps)
            r = statp.tile([1, 1], F32, tag="r")
            nc.vector.reciprocal(r, s_sb)
            r_ps = ps_small.tile([P, 1], F32, tag="ps_small")
            nc.tensor.matmul(r_ps, lhsT=ones_row, rhs=r, start=True, stop=True)
            r_col = statp.tile([P, 1], F32, tag="r_col")
            nc.scalar.copy(r_col, r_ps)
            # normalized weights in matmul layout (bf16 columns)
            W_cols = statp.tile([P, NT], BF16, tag="W_cols")
            nc.vector.tensor_scalar_mul(W_cols, P_cols, r_col)

            # weights output: transpose P_cols -> [NT, 128] rows, scale by r
            wT_ps = ps_w.tile([NT, P], F32, tag="ps_w")
            nc.tensor.transpose(wT_ps, P_cols, ident)
            w_sb = outp.tile([NT, P], F32, tag="w_sb")
            nc.scalar.activation(w_sb, wT_ps, ACTF.Copy, bias=0.0, scale=r_col[:NT])
            nc.sync.dma_start(out=w_ap[b].rearrange("(j p) -> j p", p=P), in_=w_sb)

            # ---- stage 3: context = sum_t w[t] * prod[t, :]  (then / v) ----
            ctx_ps = ps_ctx.tile([1, D], F32, tag="ctx_ps")
            for j in range(NT):
                nc.tensor.matmul(
                    ctx_ps,
                    lhsT=W_cols[:, j : j + 1],
                    rhs=prod_b[:, j, :],
                    start=(j == 0),
                    stop=(j == NT - 1),
                )
            ctx_raw = outp.tile([1, D], F32, tag="ctx_raw")
            nc.scalar.copy(ctx_raw, ctx_ps)
            ctx_sb = outp.tile([1, D], F32, tag="ctx_sb")
            nc.gpsimd.tensor_mul(ctx_sb, ctx_raw, v_inv)
            nc.sync.dma_start(out=ctx_ap[b : b + 1, :], in_=ctx_sb)


_CACHED_NC = None


def _get_nc():
    global _CACHED_NC
    if _CACHED_NC is not None:
        return _CACHED_NC
    nc = bacc.Bacc(
        "TRN2",
        target_bir_lowering=False,
        debug=False,
        enable_asserts=False,
        num_devices=NCORES,
    )
    x = nc.dram_tensor("x", [BPC, T, D], F32, kind="ExternalInput")
    v = nc.dram_tensor("v", [D, 1], F32, kind="ExternalInput")
    ctx_out = nc.dram_tensor("ctx", [BPC, D], F32, kind="ExternalOutput")
    w_out = nc.dram_tensor("w", [BPC, T], F32, kind="ExternalOutput")
    with tile.TileContext(nc) as tc:
        build_tile_kernel(tc, x.ap(), v.ap(), ctx_out.ap(), w_out.ap())
    nc.compile()
    _CACHED_NC = nc
    return nc


def _run(encoder_outputs, attn_weights_param, trace=False, **kw):
    nc = _get_nc()
    x = np.ascontiguousarray(np.asarray(encoder_outputs, dtype=np.float32))
    v = np.ascontiguousarray(np.asarray(attn_weights_param, dtype=np.float32))
    in_maps = [
        {"x": x[c * BPC : (c + 1) * BPC], "v": v} for c in range(NCORES)
    ]
    res = bass_utils.run_bass_kernel_spmd(
        nc, in_maps, core_ids=list(range(NCORES)), trace=trace, **kw
    )
    context = np.concatenate([res.results[c]["ctx"] for c in range(NCORES)], axis=0)
    weights = np.concatenate([res.results[c]["w"] for c in range(NCORES)], axis=0)
    return (context, weights), res


def kernel(encoder_outputs, attn_weights_param):
    (context, weights), _ = _run(encoder_outputs, attn_weights_param, trace=False)
    return (context, weights)


# revision 8
# speedup vs baseline: 1.1566x; 1.1566x over previous
"""Trainium2 Bass kernel for nn_Attention_67156108640667 (pooling attention).

reference:
    energies = einsum('btd,d->bt', x, v)         # GEMV per batch
    weights  = softmax(energies, axis=1)          # [B, T]
    context  = einsum('bt,btd->bd', weights, x)   # weighted-sum pool

Shapes: B=64, T=4096, D=512, f32.

Strategy (data-parallel over B across 8 NeuronCores, no collectives):
  Per core: 8 batches, each x[b] = [4096, 512] f32 (8 MiB).  Single HBM pass:
  - Stream x in 1 MiB chunks; for each [128t, 512d] tile one fused DVE
    tensor_tensor_reduce computes prod = x*v_bcast (written bf16, kept for
    the context matmul) and accum_out = energies column [128, 1].
  - Softmax on the [128, 32] energy matrix per batch (PE transposes/broadcasts
    for the cross-partition max/sum, ACT exp with accumulate).
  - Context = sum_t w[t] * prod[t, :] via PE bf16 matmuls accumulated in PSUM,
    then divided by v at the end (prod/v == x with only bf16 relative error).
"""

import os
from contextlib import ExitStack

import numpy as np

import concourse.bass as bass
import concourse.bacc as bacc
import concourse.tile as tile
from concourse import mybir
from concourse import bass_utils
from concourse.masks import make_identity

B, T, D = 64, 4096, 512
NCORES = 8
BPC = B // NCORES            # 8 batches per core
P = 128                      # SBUF partitions
NT = T // P                  # 32 t-tiles per batch
CHUNK_TILES = 8              # t-tiles per DMA chunk (8*128*512*4B = 2 MiB)
NCHUNK = NT // CHUNK_TILES   # 4 chunks per batch

F32 = mybir.dt.float32
BF16 = mybir.dt.bfloat16
ALU = mybir.AluOpType
ACTF = mybir.ActivationFunctionType
AX = mybir.AxisListType


def build_tile_kernel(tc, x_ap, v_ap, ctx_ap, w_ap):
    """Emit the per-core program.

    x_ap:   [BPC, T, D] f32 DRAM in
    v_ap:   [D, 1]      f32 DRAM in
    ctx_ap: [BPC, D]    f32 DRAM out
    w_ap:   [BPC, T]    f32 DRAM out
    """
    nc = tc.nc
    with ExitStack() as ctx:
        consts = ctx.enter_context(tc.tile_pool(name="consts", bufs=1))
        xpool = ctx.enter_context(tc.tile_pool(name="xpool", bufs=5))
        prodp = ctx.enter_context(tc.tile_pool(name="prodp", bufs=3))
        statp = ctx.enter_context(tc.tile_pool(name="statp", bufs=3))
        outp = ctx.enter_context(tc.tile_pool(name="outp", bufs=3))
        ps_small = ctx.enter_context(tc.tile_pool(name="ps_small", bufs=3, space="PSUM"))
        ps_w = ctx.enter_context(tc.tile_pool(name="ps_w", bufs=2, space="PSUM"))
        ps_ctx = ctx.enter_context(tc.tile_pool(name="ps_ctx", bufs=2, space="PSUM"))

        # ---- one-time constants ----
        ident = consts.tile([P, P], F32)
        make_identity(nc, ident)
        ident_bf = consts.tile([P, P], BF16)
        nc.vector.tensor_copy(ident_bf, ident)
        ones_row = consts.tile([1, P], F32)
        nc.vector.memset(ones_row, 1.0)
        neg_ones_row = consts.tile([1, P], F32)
        nc.vector.memset(neg_ones_row, -1.0)
        ones_col = consts.tile([P, 1], F32)
        nc.vector.memset(ones_col, 1.0)

        v_row_src = v_ap.rearrange("d one -> one d")  # [1, D] in DRAM
        v_row = consts.tile([1, D], F32)
        nc.sync.dma_start(out=v_row, in_=v_row_src)
        v_bcast = consts.tile([P, D], F32)
        nc.gpsimd.dma_start(out=v_bcast, in_=v_row_src.to_broadcast([P, D]))
        v_inv = consts.tile([1, D], F32)
        nc.vector.reciprocal(v_inv, v_row)

        for b in range(BPC):
            # ---- stage 1: stream x, fused multiply + row-reduce (energies) --
            E_cols = statp.tile([P, NT], F32, tag="E_cols")
            prod_b = prodp.tile([P, NT, D], BF16, tag="prod")
            for c in range(NCHUNK):
                xc = xpool.tile([P, CHUNK_TILES, D], F32, tag="xc")
                rows = x_ap[b, c * CHUNK_TILES * P : (c + 1) * CHUNK_TILES * P, :]
                # alternate between the two HWDGE rings (SP / ACT sequencers)
                dma_eng = nc.sync if (b * NCHUNK + c) % 2 == 0 else nc.scalar
                dma_eng.dma_start(out=xc, in_=rows.rearrange("(j p) d -> p j d", p=P))
                for jj in range(CHUNK_TILES):
                    j = c * CHUNK_TILES + jj
                    nc.vector.scalar_tensor_tensor(
                        out=prod_b[:, j, :],
                        in0=xc[:, jj, :],
                        scalar=1.0,
                        in1=v_bcast,
                        op0=ALU.mult,
                        op1=ALU.mult,
                        accum_out=E_cols[:, j : j + 1],
                    )

            # ---- stage 2: softmax over all T=4096 energies of batch b ------
            m1 = statp.tile([P, 1], F32, tag="m1")
            nc.vector.tensor_reduce(m1, E_cols, axis=AX.X, op=ALU.max)
            mT_ps = ps_small.tile([1, P], F32, tag="ps_small")
            nc.tensor.transpose(mT_ps, m1, ident)
            mT = statp.tile([1, P], F32, tag="mT")
            nc.scalar.copy(mT, mT_ps)
            # global max m, broadcast of -m to all 128 partitions via PE
            m = statp.tile([1, 1], F32, tag="m")
            nc.vector.tensor_reduce(m, mT, axis=AX.X, op=ALU.max)
            negm_ps = ps_small.tile([P, 1], F32, tag="ps_small")
            nc.tensor.matmul(negm_ps, lhsT=neg_ones_row, rhs=m, start=True, stop=True)
            negm = statp.tile([P, 1], F32, tag="negm")
            nc.scalar.copy(negm, negm_ps)
            # P_cols = exp(E - m) written directly in bf16 (matmul lhsT layout);
            # s1 = per-partition fp32 sums of the exact fp32 exp values
            P_cols = statp.tile([P, NT], BF16, tag="P_cols")
            s1 = statp.tile([P, 1], F32, tag="s1")
            nc.scalar.activation(
                P_cols, E_cols, ACTF.Exp, bias=negm, scale=1.0, accum_out=s1
            )

            # ---- stage 3: ctx_raw = sum_t exp[t] * prod[t, :] -------------
            # starts right after exp; normalization (1/s) and 1/v fold into
            # the epilogue scales.
            ctx_ps = ps_ctx.tile([1, D], F32, tag="ctx_ps")
            for j in range(NT):
                nc.tensor.matmul(
                    ctx_ps,
                    lhsT=P_cols[:, j : j + 1],
                    rhs=prod_b[:, j, :],
                    start=(j == 0),
                    stop=(j == NT - 1),
                )

            # total sum across partitions, reciprocal, broadcast back
            s_ps = ps_small.tile([1, 1], F32, tag="ps_small")
            nc.tensor.matmul(s_ps, lhsT=s1, rhs=ones_col, start=True, stop=True)
            s_sb = statp.tile([1, 1], F32, tag="s_sb")
            nc.scalar.copy(s_sb, s_ps)
            r = statp.tile([1, 1], F32, tag="r")
            nc.vector.reciprocal(r, s_sb)
            r_ps = ps_small.tile([P, 1], F32, tag="ps_small")
            nc.tensor.matmul(r_ps, lhsT=ones_row, rhs=r, start=True, stop=True)
            r_col = statp.tile([P, 1], F32, tag="r_col")
            nc.scalar.copy(r_col, r_ps)

            # weights output: transpose P_cols -> [NT, 128] rows, scale by r
            wT_ps = ps_w.tile([NT, P], BF16, tag="ps_w")
            nc.tensor.transpose(wT_ps, P_cols, ident_bf)
            w_sb = outp.tile([NT, P], F32, tag="w_sb")
            nc.scalar.activation(w_sb, wT_ps, ACTF.Copy, bias=0.0, scale=r_col[:NT])
            nc.sync.dma_start(out=w_ap[b].rearrange("(j p) -> j p", p=P), in_=w_sb)

            # context epilogue: scale by 1/s on ACT, then by 1/v on GpSimd
            ctx_raw = outp.tile([1, D], F32, tag="ctx_raw")
            nc.scalar.activation(ctx_raw, ctx_ps, ACTF.Copy, bias=0.0, scale=r)
            ctx_sb = outp.tile([1, D], F32, tag="ctx_sb")
            nc.gpsimd.tensor_mul(ctx_sb, ctx_raw, v_inv)
            nc.sync.dma_start(out=ctx_ap[b : b + 1, :], in_=ctx_sb)


_CACHED_NC = None


def _get_nc():
    global _CACHED_NC
    if _CACHED_NC is not None:
        return _CACHED_NC
    nc = bacc.Bacc(
        "TRN2",
        target_bir_lowering=False,
        debug=False,
        enable_asserts=False,
        num_devices=NCORES,
    )
    x = nc.dram_tensor("x", [BPC, T, D], F32, kind="ExternalInput")
    v = nc.dram_tensor("v", [D, 1], F32, kind="ExternalInput")
    ctx_out = nc.dram_tensor("ctx", [BPC, D], F32, kind="ExternalOutput")
    w_out = nc.dram_tensor("w", [BPC, T], F32, kind="ExternalOutput")
    with tile.TileContext(nc) as tc:
        build_tile_kernel(tc, x.ap(), v.ap(), ctx_out.ap(), w_out.ap())
    nc.compile()
    _CACHED_NC = nc
    return nc


def _run(encoder_outputs, attn_weights_param, trace=False, **kw):
    nc = _get_nc()
    x = np.ascontiguousarray(np.asarray(encoder_outputs, dtype=np.float32))
    v = np.ascontiguousarray(np.asarray(attn_weights_param, dtype=np.float32))
    in_maps = [
        {"x": x[c * BPC : (c + 1) * BPC], "v": v} for c in range(NCORES)
    ]
    res = bass_utils.run_bass_kernel_spmd(
        nc, in_maps, core_ids=list(range(NCORES)), trace=trace, **kw
    )
    context = np.concatenate([res.results[c]["ctx"] for c in range(NCORES)], axis=0)
    weights = np.concatenate([res.results[c]["w"] for c in range(NCORES)], axis=0)
    return (context, weights), res


def kernel(encoder_outputs, attn_weights_param):
    (context, weights), _ = _run(encoder_outputs, attn_weights_param, trace=False)
    return (context, weights)
